# revision 1
# baseline (speedup 1.0000x reference)
"""Chopfield attention v2 — collective-free score chain.

Key identity: Z = BETA*Re(conj(Q) @ K^T) with Q = R@W_Q, K = Y@W_K.
    Z = Re( conj(R W_Q) (Y W_K)^T ) = Re( [conj(Q) @ W_K^T] @ Y^T )
Define Q' = conj(Q) @ W_K^T  (second query-side projection, contraction
over d_out). Then Z[n,m] = Re(sum_k Q'[n,k] Y[m,k]) needs only the core's
OWN query rows plus the replicated inputs W_K and Y — the 32MB K^T
AllGather of the row-sharded design disappears; FLOPs are identical
since N == M. Only the (slack-rich) 16MB V AllGather remains.

Precision: score chain in 3-pass fp16 hi/lo (fp16 products are exact on
the PE, fp32 PSUM accumulate) — validated at rel_err 4.7e-4. V path is
single-pass fp16 Karatsuba.
"""

import numpy as np

import concourse.bacc as bacc
import concourse.mybir as mybir
import concourse.tile as tile
from concourse.bass_utils import run_bass_kernel_spmd

BETA = 0.03125
P = 128
FP16 = mybir.dt.float16
FP32 = mybir.dt.float32
X = mybir.AxisListType.X


class Cfg:
    def __init__(self, N=4096, M=4096, D=1024, NC=8):
        self.N, self.M, self.D, self.NC = N, M, D, NC
        self.NL = N // NC          # local query rows
        self.ML = M // NC          # local key rows (V path)
        self.DT = D // P           # contraction tiles
        self.QTS = self.NL // P    # local query partition-tiles
        self.MTS = self.ML // P    # local key partition-tiles
        self.DF = min(512, D)      # free-dim chunk for D-wide outputs
        self.DCH = D // self.DF
        self.MTG = M // P          # global key partition-tiles
        self.MF = 512              # score key-chunk width
        self.MCH = M // self.MF    # score key chunks
        self.SLOT = D * self.ML    # elements per gathered V tensor slot


def build(cfg: Cfg, reps: int = 1, no_collective: bool = False, stop_after: str | None = None):
    c = cfg
    nc = bacc.Bacc("TRN2", target_bir_lowering=False, debug=False, num_devices=c.NC)

    def din(name, shape, dt=FP16):
        return nc.dram_tensor(name, shape, dt, kind="ExternalInput")

    # V path: local Y^T shard (h only) + V weights
    ytl = {n: din(f"ytl_{n}", [P, c.DT * c.ML]) for n in ("re", "im", "s")}
    wv = {n: din(f"wv_{n}", [c.DCH, P, c.DT * c.DF]) for n in ("re", "im", "s")}
    # Q path: local R^T shard hi/lo + Q weights hi/lo (BETA folded)
    rt = {}
    wq = {}
    wk2 = {}
    for comp in ("re", "im", "s"):
        for lvl in ("h", "l"):
            rt[comp, lvl] = din(f"rt_{comp}_{lvl}", [P, c.DT * c.NL])
            wq[comp, lvl] = din(f"wq_{comp}_{lvl}", [c.DT, P, c.DT * P])
            wk2[comp, lvl] = din(f"wk2_{comp}_{lvl}", [c.DT, P, c.DT * P])
    # score moving operand: FULL Y^T hi/lo (replicated input)
    ytf = {}
    for comp in ("re", "im"):
        for lvl in ("h", "l"):
            ytf[comp, lvl] = din(f"ytf_{comp}_{lvl}", [c.MCH, P, c.DT * c.MF])

    ident = din("ident", [P, P])

    o_re = nc.dram_tensor("o_re", [c.NL, c.D], FP32, kind="ExternalOutput")
    o_im = nc.dram_tensor("o_im", [c.NL, c.D], FP32, kind="ExternalOutput")

    with tile.TileContext(nc) as tc:
        with (
            tc.tile_pool(name="pers", bufs=1) as pers,
            tc.tile_pool(name="ps", bufs=1, space="PSUM") as ps,
            tc.tile_pool(name="dram", bufs=1, space="DRAM") as dram,
        ):
            def emit(rep):
                # L pool: 12 slots of [P, DT*NL] f16, aliased through phases:
                #   rt tags: R^T loads -> q2_sb (Q'proj out) -> pt_sb (transposed A)
                #   qt tags: qt_sb (Qproj out) -> p_sb (softmax probs)
                L = tc.alloc_tile_pool(name=f"L{rep}", bufs=1)

                def Lt(tag):
                    return L.tile([P, c.DT * c.NL], FP16, tag=tag, name=f"{tag}_t{rep}")

                ident_sb = pers.tile([P, P], FP16, tag="ident")
                nc.sync.dma_start(ident_sb[:], ident.ap())
                cm = [L.tile([P, c.MCH], FP32, tag=f"cm{qt}", name=f"cm{qt}_{rep}") for qt in range(c.QTS)]
                ncm = [L.tile([P, c.MCH], FP32, tag=f"ncm{qt}", name=f"ncm{qt}_{rep}") for qt in range(c.QTS)]
                recip = [L.tile([P, 1], FP32, tag=f"rcp{qt}", name=f"rcp{qt}_{rep}") for qt in range(c.QTS)]

                agv_in = dram.tile([2 * c.SLOT], FP16)
                agv_out = dram.tile([c.NC * 2 * c.SLOT], FP16, addr_space="Shared")

                # ---------- V projection (single-pass fp16 Karatsuba) + AG(V) early
                kvp = tc.alloc_tile_pool(name=f"kvp{rep}", bufs=1)
                ytls = {}
                for n, t in ytl.items():
                    ytls[n] = kvp.tile([P, c.DT * c.ML], FP16, tag=f"ytl{n}", name=f"ytl{n}_{rep}")
                    nc.scalar.dma_start(ytls[n][:], t.ap())
                rts = {}
                for (comp, lvl), t in rt.items():
                    rts[comp, lvl] = Lt(f"rt_{comp}_{lvl}")
                    nc.scalar.dma_start(rts[comp, lvl][:], t.ap())
                vp = tc.alloc_tile_pool(name=f"vp{rep}", bufs=1)
                wvidx = {"re": 0, "im": 1, "s": 2}
                for dch in range(c.DCH):
                    wvsl = vp.tile([P, 3 * c.DT * c.DF], FP16, tag="wvsl", bufs=2)
                    for wn, wi in wvidx.items():
                        nc.sync.dma_start(
                            wvsl[:, wi * c.DT * c.DF : (wi + 1) * c.DT * c.DF],
                            wv[wn].ap()[dch],
                        )
                    for mt in range(c.MTS):
                        m = {}
                        for prod, yc in enumerate(("re", "im", "s")):
                            pt = ps.tile([P, 512], FP32, tag="ps", bufs=6)
                            m[prod] = pt[:, : c.DF]
                            for ki in range(c.DT):
                                nc.tensor.matmul(
                                    m[prod],
                                    ytls[yc][:, ki * c.ML + mt * P : ki * c.ML + (mt + 1) * P],
                                    wvsl[:, wvidx[yc] * c.DT * c.DF + ki * c.DF : wvidx[yc] * c.DT * c.DF + (ki + 1) * c.DF],
                                    start=(ki == 0),
                                    stop=(ki == c.DT - 1),
                                )
                        vm2s = vp.tile([P, c.DF], FP32, tag="vm2s", bufs=2)
                        nc.vector.tensor_copy(vm2s[:], m[1])
                        for comp, si in (("re", 0), ("im", 1)):
                            vout = vp.tile([P, c.DF], FP16, tag="vout", bufs=4)
                            if comp == "re":
                                nc.vector.tensor_sub(vout[:], m[0], vm2s[:])
                            else:
                                vim1 = vp.tile([P, c.DF], FP32, tag="vim1", bufs=2)
                                nc.vector.tensor_sub(vim1[:], m[2], vm2s[:])
                                nc.vector.tensor_sub(vout[:], vim1[:], m[0])
                            dst = agv_in[si * c.SLOT : (si + 1) * c.SLOT].rearrange(
                                "(m p dc d) -> m p dc d", m=c.MTS, p=P, dc=c.DCH
                            )[mt, :, dch, :]
                            nc.gpsimd.dma_start(dst, vout[:])
                if not no_collective:
                    nc.gpsimd.collective_compute(
                        "AllGather",
                        mybir.AluOpType.bypass,
                        replica_groups=[list(range(c.NC))],
                        ins=[agv_in.opt()],
                        outs=[agv_out.opt()],
                    )
                vp.release()
                kvp.release()
                if stop_after == "vproj":
                    L.release()
                    return

                # ---------- Q projection: Q^T = W_Q^T @ R^T (3-pass Karatsuba)
                # outputs re/im/s hi+lo (s = re+im, feeds Q'proj Karatsuba)
                qt_sb = {}
                for comp in ("re", "im", "s"):
                    for lvl in ("h", "l"):
                        qt_sb[comp, lvl] = Lt(f"qt_{comp}_{lvl}")
                widx = {("re", "h"): 0, ("re", "l"): 1, ("im", "h"): 2,
                        ("im", "l"): 3, ("s", "h"): 4, ("s", "l"): 5}
                qp = tc.alloc_tile_pool(name=f"qp{rep}", bufs=1)
                for dt_out in range(c.DT):
                    wsl = qp.tile([P, 6 * c.DT * P], FP16, tag="wqsl", bufs=2)
                    for (wc, wl), wi in widx.items():
                        nc.sync.dma_start(
                            wsl[:, wi * c.DT * P : (wi + 1) * c.DT * P],
                            wq[wc, wl].ap()[dt_out],
                        )

                    def wslice(wc, wl, ki):
                        wi = widx[wc, wl]
                        return wsl[:, wi * c.DT * P + ki * P : wi * c.DT * P + (ki + 1) * P]

                    m = {}
                    for prod, comp in enumerate(("re", "im", "s")):
                        pt = ps.tile([P, 512], FP32, tag="ps", bufs=6)
                        m[prod] = pt[:, : c.NL]
                        nmm = c.DT * 3
                        i = 0
                        for ki in range(c.DT):
                            for wl, ml in (("h", "h"), ("h", "l"), ("l", "h")):
                                nc.tensor.matmul(
                                    m[prod],
                                    wslice(comp, wl, ki),
                                    rts[comp, ml][:, ki * c.NL : (ki + 1) * c.NL],
                                    start=(i == 0),
                                    stop=(i == nmm - 1),
                                )
                                i += 1
                    # re = m1 - m2 ; im = m3 - m1 - m2 ; s = re + im
                    m2s = qp.tile([P, c.NL], FP32, tag="qm2s", bufs=2)
                    nc.vector.tensor_copy(m2s[:], m[1])
                    dre = qp.tile([P, c.NL], FP32, tag="qdre", bufs=2)
                    nc.vector.tensor_sub(dre[:], m[0], m2s[:])
                    dim = qp.tile([P, c.NL], FP32, tag="qdim", bufs=2)
                    nc.vector.tensor_sub(dim[:], m[2], m2s[:])
                    nc.vector.tensor_sub(dim[:], dim[:], m[0])
                    dsum = qp.tile([P, c.NL], FP32, tag="qdsum", bufs=2)
                    nc.vector.tensor_add(dsum[:], dre[:], dim[:])
                    for comp, d in (("re", dre), ("im", dim), ("s", dsum)):
                        hi = qt_sb[comp, "h"][:, dt_out * c.NL : (dt_out + 1) * c.NL]
                        lo = qt_sb[comp, "l"][:, dt_out * c.NL : (dt_out + 1) * c.NL]
                        nc.vector.tensor_copy(hi, d[:])
                        nc.vector.tensor_sub(lo, d[:], hi)
                qp.release()
                if stop_after == "qproj":
                    L.release()
                    return

                # ---------- Q' projection: Q'^T = W_K @ conj(Q)^T
                # m1 = a@c, m2 = b@d, m3 = (a+b)@(d-c); a=Qre, b=Qim,
                # c=Re(Wk^T), d=Im(Wk^T); re(Q') = m1+m2, -im(Q') = m2-m1-m3
                q2_sb = {}
                for comp, lvl in ((x, y) for x in ("re", "nim") for y in ("h", "l")):
                    src_tag = {"re": "rt_re", "nim": "rt_im"}[comp]
                    q2_sb[comp, lvl] = Lt(f"{src_tag}_{lvl}")
                q2p = tc.alloc_tile_pool(name=f"q2p{rep}", bufs=1)
                for ko in range(c.DT):
                    wsl = q2p.tile([P, 6 * c.DT * P], FP16, tag="wksl", bufs=2)
                    for (wc, wl), wi in widx.items():
                        nc.sync.dma_start(
                            wsl[:, wi * c.DT * P : (wi + 1) * c.DT * P],
                            wk2[wc, wl].ap()[ko],
                        )

                    def wslice2(wc, wl, ki):
                        wi = widx[wc, wl]
                        return wsl[:, wi * c.DT * P + ki * P : wi * c.DT * P + (ki + 1) * P]

                    m = {}
                    for prod, comp in enumerate(("re", "im", "s")):
                        pt = ps.tile([P, 512], FP32, tag="ps", bufs=6)
                        m[prod] = pt[:, : c.NL]
                        nmm = c.DT * 3
                        i = 0
                        for ki in range(c.DT):
                            for wl, ml in (("h", "h"), ("h", "l"), ("l", "h")):
                                nc.tensor.matmul(
                                    m[prod],
                                    wslice2(comp, wl, ki),
                                    qt_sb[comp, ml][:, ki * c.NL : (ki + 1) * c.NL],
                                    start=(i == 0),
                                    stop=(i == nmm - 1),
                                )
                                i += 1
                    m2s = q2p.tile([P, c.NL], FP32, tag="km2s", bufs=2)
                    nc.vector.tensor_copy(m2s[:], m[1])
                    dre = q2p.tile([P, c.NL], FP32, tag="kdre", bufs=2)
                    nc.vector.tensor_add(dre[:], m[0], m2s[:])
                    dnim = q2p.tile([P, c.NL], FP32, tag="kdnim", bufs=2)
                    nc.vector.tensor_sub(dnim[:], m2s[:], m[0])
                    nc.vector.tensor_sub(dnim[:], dnim[:], m[2])
                    for comp, d in (("re", dre), ("nim", dnim)):
                        hi = q2_sb[comp, "h"][:, ko * c.NL : (ko + 1) * c.NL]
                        lo = q2_sb[comp, "l"][:, ko * c.NL : (ko + 1) * c.NL]
                        nc.vector.tensor_copy(hi, d[:])
                        nc.vector.tensor_sub(lo, d[:], hi)
                q2p.release()
                if stop_after == "q2proj":
                    L.release()
                    return

                # ---------- scores + streaming softmax over key chunks
                # Z[q, m] = sum_k Q're[q,k] Yre[m,k] + Q'nim[q,k] Yim[m,k]
                p_sb = [Lt(f"qt_{comp}_{lvl}") for comp, lvl in
                        (("re", "h"), ("re", "l"), ("im", "h"), ("im", "l"))]
                scp = tc.alloc_tile_pool(name=f"scp{rep}", bufs=1)
                ysl_idx = {("re", "h"): 0, ("re", "l"): 1, ("im", "h"): 2, ("im", "l"): 3}
                for mch in range(c.MCH):
                    ytile = scp.tile([P, 4 * c.DT * c.MF], FP16, tag="ytile", bufs=2)
                    for (yc, yl), yi in ysl_idx.items():
                        nc.scalar.dma_start(
                            ytile[:, yi * c.DT * c.MF : (yi + 1) * c.DT * c.MF],
                            ytf[yc, yl].ap()[mch],
                        )

                    def yslice(yc, yl, ki):
                        yi = ysl_idx[yc, yl]
                        return ytile[:, yi * c.DT * c.MF + ki * c.MF : yi * c.DT * c.MF + (ki + 1) * c.MF]

                    for qt in range(c.QTS):
                        zp = ps.tile([P, 512], FP32, tag="ps", bufs=6)
                        zacc = zp[:, : c.MF]
                        nmm = 2 * c.DT * 3
                        i = 0
                        for comp, yc in (("re", "re"), ("nim", "im")):
                            for ki in range(c.DT):
                                for ql, yl in (("h", "h"), ("h", "l"), ("l", "h")):
                                    nc.tensor.matmul(
                                        zacc,
                                        q2_sb[comp, ql][:, ki * c.NL + qt * P : ki * c.NL + (qt + 1) * P],
                                        yslice(yc, yl, ki),
                                        start=(i == 0),
                                        stop=(i == nmm - 1),
                                    )
                                    i += 1
                        nc.vector.reduce_max(cm[qt][:, mch : mch + 1], zacc, axis=X)
                        nc.vector.tensor_scalar_mul(
                            ncm[qt][:, mch : mch + 1], cm[qt][:, mch : mch + 1], -1.0
                        )
                        nc.scalar.activation(
                            p_sb[qt][:, mch * c.MF : (mch + 1) * c.MF],
                            zacc,
                            mybir.ActivationFunctionType.Exp,
                            bias=ncm[qt][:, mch : mch + 1],
                            scale=1.0,
                        )

                # ---------- finalize softmax: rescale chunks to global max
                for qt in range(c.QTS):
                    ngm = scp.tile([P, 1], FP32, tag=f"ngm{qt}", name=f"ngm{qt}_{rep}")
                    nc.vector.tensor_reduce(
                        ngm[:], ncm[qt][:], op=mybir.AluOpType.min, axis=X
                    )
                    fac = scp.tile([P, c.MCH], FP32, tag=f"fac{qt}", name=f"fac{qt}_{rep}")
                    nc.scalar.activation(
                        fac[:],
                        ncm[qt][:],
                        mybir.ActivationFunctionType.Exp,
                        bias=ngm[:, 0:1],
                        scale=-1.0,
                    )
                    for mch in range(c.MCH):
                        nc.vector.tensor_scalar_mul(
                            p_sb[qt][:, mch * c.MF : (mch + 1) * c.MF],
                            p_sb[qt][:, mch * c.MF : (mch + 1) * c.MF],
                            fac[:, mch : mch + 1],
                        )
                    ssum = scp.tile([P, 1], FP32, tag=f"ssum{qt}")
                    nc.vector.reduce_sum(ssum[:], p_sb[qt][:], axis=X)
                    nc.vector.reciprocal(recip[qt][:], ssum[:])
                scp.release()
                if stop_after == "scores":
                    L.release()
                    return

                # ---------- transpose P -> P^T tiles ([m-part, q-free])
                # pt_sb aliases the rt/q2 slots: 4 flat tiles, 8 mtg-chunks each
                pt_flat = [Lt(f"rt_{comp}_{lvl}") for comp, lvl in
                           (("re", "h"), ("re", "l"), ("im", "h"), ("im", "l"))]

                def pt_slice(mtg, qt=None):
                    base = (mtg % 8) * c.NL
                    if qt is None:
                        return pt_flat[mtg // 8][:, base : base + c.NL]
                    return pt_flat[mtg // 8][:, base + qt * P : base + (qt + 1) * P]

                avp = tc.alloc_tile_pool(name=f"avp{rep}", bufs=1)
                for mtg in range(c.MTG):
                    tp = ps.tile([P, 512], FP16, tag="dsc", bufs=2)
                    tacc = tp[:, : c.NL]
                    for qt in range(c.QTS):
                        nc.tensor.matmul(
                            tacc[:, qt * P : (qt + 1) * P],
                            p_sb[qt][:, mtg * P : (mtg + 1) * P],
                            ident_sb[:],
                            start=True,
                            stop=True,
                            is_transpose=True,
                        )
                    nc.vector.tensor_copy(pt_slice(mtg), tacc)
                if stop_after == "transp":
                    avp.release()
                    L.release()
                    return

                # ---------- A @ V (+ 1/sum scaling)
                for comp, odram in (("re", o_re), ("im", o_im)):
                    si = 0 if comp == "re" else 1
                    for dch in range(c.DCH):
                        vh = avp.tile([P, c.MTG * c.DF], FP16, tag="vh", bufs=2)
                        for r in range(c.NC):
                            src = agv_out[
                                r * 2 * c.SLOT + si * c.SLOT : r * 2 * c.SLOT + (si + 1) * c.SLOT
                            ].rearrange("(m p dc d) -> dc p m d", m=c.MTS, p=P, dc=c.DCH)[dch]
                            nc.sync.dma_start(
                                vh[
                                    :, r * c.MTS * c.DF : (r + 1) * c.MTS * c.DF
                                ].rearrange("p (m d) -> p m d", m=c.MTS),
                                src,
                            )
                        for qt in range(c.QTS):
                            op_ = ps.tile([P, 512], FP32, tag="ps", bufs=6)
                            oacc = op_[:, : c.DF]
                            for mtg in range(c.MTG):
                                nc.tensor.matmul(
                                    oacc,
                                    pt_slice(mtg, qt),
                                    vh[:, mtg * c.DF : (mtg + 1) * c.DF],
                                    start=(mtg == 0),
                                    stop=(mtg == c.MTG - 1),
                                )
                            osb = avp.tile([P, c.DF], FP32, tag="osb", bufs=4)
                            nc.vector.tensor_scalar_mul(osb[:], oacc, recip[qt][:, 0:1])
                            nc.sync.dma_start(
                                odram.ap()[
                                    qt * P : (qt + 1) * P, dch * c.DF : (dch + 1) * c.DF
                                ],
                                osb[:],
                            )
                avp.release()
                L.release()

            for rep in range(reps):
                emit(rep)

    nc.compile()
    return nc


def _split16(x):
    h = x.astype(np.float16)
    l = (x - h.astype(np.float32)).astype(np.float16)
    return h, l


def prep_inputs(cfg, R_re, R_im, Y_re, Y_im, W_Q_re, W_Q_im, W_K_re, W_K_im, W_V_re, W_V_im):
    """Host-side sharding + fp16 hi/lo split + transposes. Returns in_maps."""
    c = cfg
    f32 = np.float32
    DT, DCH, DF, MCH, MF = c.DT, c.DCH, c.DF, c.MCH, c.MF

    def _wsw(w16, ocols):
        # [d_in, d_out] -> [d_out_block, p, d_in_tile * ocols], contiguous
        ob = w16.shape[1] // ocols
        return np.ascontiguousarray(
            w16.reshape(DT, P, ob, ocols).transpose(2, 1, 0, 3).reshape(ob, P, DT * ocols)
        )

    wq_re = np.ascontiguousarray(W_Q_re, dtype=f32) * BETA
    wq_im = np.ascontiguousarray(W_Q_im, dtype=f32) * BETA
    wk_re = np.ascontiguousarray(W_K_re, dtype=f32)
    wk_im = np.ascontiguousarray(W_K_im, dtype=f32)
    wv_re = np.ascontiguousarray(W_V_re, dtype=f32)
    wv_im = np.ascontiguousarray(W_V_im, dtype=f32)

    shared = {}
    wqs = {"re": _split16(wq_re), "im": _split16(wq_im), "s": _split16(wq_re + wq_im)}
    # Q'proj stationary: c=Wk_re^T, d=Wk_im^T, e=d-c, in [j, k] layout
    c_mat = np.ascontiguousarray(wk_re.T)
    d_mat = np.ascontiguousarray(wk_im.T)
    wk2s = {"re": _split16(c_mat), "im": _split16(d_mat), "s": _split16(d_mat - c_mat)}
    for comp in ("re", "im", "s"):
        for li, lvl in enumerate(("h", "l")):
            shared[f"wq_{comp}_{lvl}"] = _wsw(wqs[comp][li], P)
            shared[f"wk2_{comp}_{lvl}"] = _wsw(wk2s[comp][li], P)
    shared["wv_re"] = _wsw(wv_re.astype(np.float16), DF)
    shared["wv_im"] = _wsw(wv_im.astype(np.float16), DF)
    shared["wv_s"] = _wsw((wv_re + wv_im).astype(np.float16), DF)
    shared["ident"] = np.eye(P, dtype=np.float16)

    # full Y^T hi/lo for scores: [MCH, P, DT*MF]
    for comp, arr in (("re", Y_re), ("im", Y_im)):
        t = np.ascontiguousarray(np.asarray(arr, dtype=f32).T)
        for lvl, a in zip(("h", "l"), _split16(t)):
            shared[f"ytf_{comp}_{lvl}"] = np.ascontiguousarray(
                a.reshape(DT, P, MCH, MF).transpose(2, 1, 0, 3).reshape(MCH, P, DT * MF)
            )

    in_maps = []
    for r in range(c.NC):
        m = dict(shared)
        rsl = slice(r * c.NL, (r + 1) * c.NL)
        ysl = slice(r * c.ML, (r + 1) * c.ML)
        rre_t = np.ascontiguousarray(np.asarray(R_re[rsl], dtype=f32).T)
        rim_t = np.ascontiguousarray(np.asarray(R_im[rsl], dtype=f32).T)
        yre_t = np.ascontiguousarray(np.asarray(Y_re[ysl], dtype=f32).T)
        yim_t = np.ascontiguousarray(np.asarray(Y_im[ysl], dtype=f32).T)
        for base, arr in (("rt_re", rre_t), ("rt_im", rim_t), ("rt_s", rre_t + rim_t)):
            h, l = _split16(arr)
            mw = arr.shape[1]
            for lvl, a in (("h", h), ("l", l)):
                m[f"{base}_{lvl}"] = np.ascontiguousarray(
                    a.reshape(DT, P, mw).transpose(1, 0, 2).reshape(P, DT * mw)
                )
        for n, arr in (("re", yre_t), ("im", yim_t), ("s", yre_t + yim_t)):
            mw = arr.shape[1]
            m[f"ytl_{n}"] = np.ascontiguousarray(
                arr.astype(np.float16).reshape(DT, P, mw).transpose(1, 0, 2).reshape(P, DT * mw)
            )
        in_maps.append(m)
    return in_maps


_NC_CACHE = {}


def kernel(**inputs) -> np.ndarray:
    cfg = Cfg()
    if "full" not in _NC_CACHE:
        _NC_CACHE["full"] = build(cfg, 1)
    nc = _NC_CACHE["full"]
    in_maps = prep_inputs(cfg, **inputs)
    res = run_bass_kernel_spmd(nc, in_maps, list(range(cfg.NC)))
    o_re = np.concatenate([res.results[r]["o_re"] for r in range(cfg.NC)], axis=0)
    o_im = np.concatenate([res.results[r]["o_im"] for r in range(cfg.NC)], axis=0)
    return (o_re + 1j * o_im).astype(np.complex64)



# revision 2
# speedup vs baseline: 5.8319x; 5.8319x over previous
"""Chopfield attention v3 — host-fused query projection.

Key identity: Z = BETA*Re(conj(Q) @ K^T) with Q = R@W_Q, K = Y@W_K.
    Z = Re( conj(R) [BETA*conj(W_Q) W_K^T] Y^T ) = Re( Q'' @ Y^T ),
    Q'' = conj(R) @ G,  G = BETA*conj(W_Q) @ W_K^T  (precomputed on host).
The weight-only product G fuses the two query-side projections of v2
into one device-side projection — the PE-cycle floor drops ~17% and the
W_K weight stream disappears. Scores need only the core's own R rows
plus replicated G and Y; the (slack-rich) 16MB V AllGather remains.

Precision: score chain in 3-pass fp16 hi/lo (fp16 products are exact on
the PE, fp32 PSUM accumulate) — validated at rel_err 4.7e-4. V path is
single-pass fp16 Karatsuba.
"""

import numpy as np

import concourse.bacc as bacc
import concourse.mybir as mybir
import concourse.tile as tile
from concourse.bass_utils import run_bass_kernel_spmd

BETA = 0.03125
P = 128
FP16 = mybir.dt.float16
FP32 = mybir.dt.float32
X = mybir.AxisListType.X


class Cfg:
    def __init__(self, N=4096, M=4096, D=1024, NC=8):
        self.N, self.M, self.D, self.NC = N, M, D, NC
        self.NL = N // NC          # local query rows
        self.ML = M // NC          # local key rows (V path)
        self.DT = D // P           # contraction tiles
        self.QTS = self.NL // P    # local query partition-tiles
        self.MTS = self.ML // P    # local key partition-tiles
        self.DF = min(512, D)      # free-dim chunk for D-wide outputs
        self.DCH = D // self.DF
        self.MTG = M // P          # global key partition-tiles
        self.MF = 512              # score key-chunk width
        self.MCH = M // self.MF    # score key chunks
        self.SLOT = D * self.ML    # elements per gathered V tensor slot


def build(cfg: Cfg, reps: int = 1, no_collective: bool = False, stop_after: str | None = None):
    c = cfg
    nc = bacc.Bacc("TRN2", target_bir_lowering=False, debug=False, num_devices=c.NC)

    def din(name, shape, dt=FP16):
        return nc.dram_tensor(name, shape, dt, kind="ExternalInput")

    # V path: local Y^T shard (h only) + V weights
    ytl = {n: din(f"ytl_{n}", [P, c.DT * c.ML]) for n in ("re", "im", "s")}
    wv = {n: din(f"wv_{n}", [c.DCH, P, c.DT * c.DF]) for n in ("re", "im", "s")}
    # Q'' path: local R^T shard hi/lo + fused G weights hi/lo (BETA folded)
    rt = {}
    g = {}
    for comp in ("re", "im", "s"):
        for lvl in ("h", "l"):
            rt[comp, lvl] = din(f"rt_{comp}_{lvl}", [P, c.DT * c.NL])
    for comp in ("re", "im", "d"):
        for lvl in ("h", "l"):
            g[comp, lvl] = din(f"g_{comp}_{lvl}", [c.DT, P, c.DT * P])
    # score moving operand: FULL Y^T hi/lo (replicated input)
    ytf = {}
    for comp in ("re", "im"):
        for lvl in ("h", "l"):
            ytf[comp, lvl] = din(f"ytf_{comp}_{lvl}", [c.MCH, P, c.DT * c.MF])

    ident = din("ident", [P, P])

    o_re = nc.dram_tensor("o_re", [c.NL, c.D], FP32, kind="ExternalOutput")
    o_im = nc.dram_tensor("o_im", [c.NL, c.D], FP32, kind="ExternalOutput")

    with tile.TileContext(nc) as tc:
        with (
            tc.tile_pool(name="pers", bufs=1) as pers,
            tc.tile_pool(name="ps", bufs=1, space="PSUM") as ps,
            tc.tile_pool(name="dram", bufs=1, space="DRAM") as dram,
        ):
            def emit(rep):
                # L pool: 10 slots of [P, DT*NL] f16, aliased through phases:
                #   rt_re/im tags: R^T loads -> p_sb (softmax probs)
                #   rt_s tags: R^T sum loads (die after Q''proj)
                #   q tags: q2_sb (Q''proj out) -> pt_sb (transposed A, 2 of 4)
                L = tc.alloc_tile_pool(name=f"L{rep}", bufs=1)

                def Lt(tag):
                    return L.tile([P, c.DT * c.NL], FP16, tag=tag, name=f"{tag}_t{rep}")

                ident_sb = pers.tile([P, P], FP16, tag="ident")
                nc.sync.dma_start(ident_sb[:], ident.ap())
                cm = [L.tile([P, c.MCH], FP32, tag=f"cm{qt}", name=f"cm{qt}_{rep}") for qt in range(c.QTS)]
                ncm = [L.tile([P, c.MCH], FP32, tag=f"ncm{qt}", name=f"ncm{qt}_{rep}") for qt in range(c.QTS)]
                recip = [L.tile([P, 1], FP32, tag=f"rcp{qt}", name=f"rcp{qt}_{rep}") for qt in range(c.QTS)]

                agv_in = dram.tile([2 * c.SLOT], FP16)
                agv_out = dram.tile([c.NC * 2 * c.SLOT], FP16, addr_space="Shared")

                # ---------- V projection (single-pass fp16 Karatsuba) + AG(V) early
                kvp = tc.alloc_tile_pool(name=f"kvp{rep}", bufs=1)
                ytls = {}
                for n, t in ytl.items():
                    ytls[n] = kvp.tile([P, c.DT * c.ML], FP16, tag=f"ytl{n}", name=f"ytl{n}_{rep}")
                    nc.scalar.dma_start(ytls[n][:], t.ap())
                rts = {}
                for (comp, lvl), t in rt.items():
                    rts[comp, lvl] = Lt(f"rt_{comp}_{lvl}")
                    nc.scalar.dma_start(rts[comp, lvl][:], t.ap())
                vp = tc.alloc_tile_pool(name=f"vp{rep}", bufs=1)
                wvidx = {"re": 0, "im": 1, "s": 2}
                for dch in range(c.DCH):
                    wvsl = vp.tile([P, 3 * c.DT * c.DF], FP16, tag="wvsl", bufs=2)
                    for wn, wi in wvidx.items():
                        nc.sync.dma_start(
                            wvsl[:, wi * c.DT * c.DF : (wi + 1) * c.DT * c.DF],
                            wv[wn].ap()[dch],
                        )
                    for mt in range(c.MTS):
                        m = {}
                        for prod, yc in enumerate(("re", "im", "s")):
                            pt = ps.tile([P, 512], FP32, tag="ps", bufs=6)
                            m[prod] = pt[:, : c.DF]
                            for ki in range(c.DT):
                                nc.tensor.matmul(
                                    m[prod],
                                    ytls[yc][:, ki * c.ML + mt * P : ki * c.ML + (mt + 1) * P],
                                    wvsl[:, wvidx[yc] * c.DT * c.DF + ki * c.DF : wvidx[yc] * c.DT * c.DF + (ki + 1) * c.DF],
                                    start=(ki == 0),
                                    stop=(ki == c.DT - 1),
                                )
                        vm2s = vp.tile([P, c.DF], FP32, tag="vm2s", bufs=2)
                        nc.vector.tensor_copy(vm2s[:], m[1])
                        for comp, si in (("re", 0), ("im", 1)):
                            vout = vp.tile([P, c.DF], FP16, tag="vout", bufs=4)
                            if comp == "re":
                                nc.vector.tensor_sub(vout[:], m[0], vm2s[:])
                            else:
                                vim1 = vp.tile([P, c.DF], FP32, tag="vim1", bufs=2)
                                nc.vector.tensor_sub(vim1[:], m[2], vm2s[:])
                                nc.vector.tensor_sub(vout[:], vim1[:], m[0])
                            dst = agv_in[si * c.SLOT : (si + 1) * c.SLOT].rearrange(
                                "(m p dc d) -> m p dc d", m=c.MTS, p=P, dc=c.DCH
                            )[mt, :, dch, :]
                            nc.gpsimd.dma_start(dst, vout[:])
                if not no_collective:
                    nc.gpsimd.collective_compute(
                        "AllGather",
                        mybir.AluOpType.bypass,
                        replica_groups=[list(range(c.NC))],
                        ins=[agv_in.opt()],
                        outs=[agv_out.opt()],
                    )
                vp.release()
                kvp.release()
                if stop_after == "vproj":
                    L.release()
                    return

                # ---------- Q'' projection: Q''^T = G^T @ conj(R)^T (3-pass Karatsuba)
                # m1 = Rre@Gre, m2 = Rim@Gim, m3 = (Rre+Rim)@(Gre-Gim)
                # re(Q'') = m1+m2 ; -im(Q'') = m3-m1+m2
                q2_sb = {}
                for comp in ("re", "nim"):
                    for lvl in ("h", "l"):
                        q2_sb[comp, lvl] = Lt(f"q_{comp}_{lvl}")
                gidx = {("re", "h"): 0, ("re", "l"): 1, ("im", "h"): 2,
                        ("im", "l"): 3, ("d", "h"): 4, ("d", "l"): 5}
                rsel = {"re": "re", "im": "im", "d": "s"}
                qp = tc.alloc_tile_pool(name=f"qp{rep}", bufs=1)
                for ko in range(c.DT):
                    wsl = qp.tile([P, 6 * c.DT * P], FP16, tag="gsl", bufs=2)
                    for (wc, wl), wi in gidx.items():
                        nc.sync.dma_start(
                            wsl[:, wi * c.DT * P : (wi + 1) * c.DT * P],
                            g[wc, wl].ap()[ko],
                        )

                    def wslice(wc, wl, ki):
                        wi = gidx[wc, wl]
                        return wsl[:, wi * c.DT * P + ki * P : wi * c.DT * P + (ki + 1) * P]

                    m = {}
                    for prod, comp in enumerate(("re", "im", "d")):
                        pt = ps.tile([P, 512], FP32, tag="ps", bufs=6)
                        m[prod] = pt[:, : c.NL]
                        nmm = c.DT * 3
                        i = 0
                        for ki in range(c.DT):
                            for wl, ml in (("h", "h"), ("h", "l"), ("l", "h")):
                                nc.tensor.matmul(
                                    m[prod],
                                    wslice(comp, wl, ki),
                                    rts[rsel[comp], ml][:, ki * c.NL : (ki + 1) * c.NL],
                                    start=(i == 0),
                                    stop=(i == nmm - 1),
                                )
                                i += 1
                    # re = m1 + m2 ; nim = m3 - m1 + m2
                    m2s = qp.tile([P, c.NL], FP32, tag="qm2s", bufs=2)
                    nc.vector.tensor_copy(m2s[:], m[1])
                    dre = qp.tile([P, c.NL], FP32, tag="qdre", bufs=2)
                    nc.vector.tensor_add(dre[:], m[0], m2s[:])
                    dnim = qp.tile([P, c.NL], FP32, tag="qdnim", bufs=2)
                    nc.vector.tensor_add(dnim[:], m[2], m2s[:])
                    nc.vector.tensor_sub(dnim[:], dnim[:], m[0])
                    for comp, d in (("re", dre), ("nim", dnim)):
                        hi = q2_sb[comp, "h"][:, ko * c.NL : (ko + 1) * c.NL]
                        lo = q2_sb[comp, "l"][:, ko * c.NL : (ko + 1) * c.NL]
                        nc.vector.tensor_copy(hi, d[:])
                        nc.vector.tensor_sub(lo, d[:], hi)
                qp.release()
                if stop_after == "qproj":
                    L.release()
                    return

                # ---------- scores + streaming softmax over key chunks
                # Z[q, m] = sum_k Q''re[q,k] Yre[m,k] + Q''nim[q,k] Yim[m,k]
                p_sb = [Lt(f"rt_{comp}_{lvl}") for comp, lvl in
                        (("re", "h"), ("re", "l"), ("im", "h"), ("im", "l"))]
                scp = tc.alloc_tile_pool(name=f"scp{rep}", bufs=1)
                ysl_idx = {("re", "h"): 0, ("re", "l"): 1, ("im", "h"): 2, ("im", "l"): 3}
                for mch in range(c.MCH):
                    ytile = scp.tile([P, 4 * c.DT * c.MF], FP16, tag="ytile", bufs=2)
                    for (yc, yl), yi in ysl_idx.items():
                        nc.scalar.dma_start(
                            ytile[:, yi * c.DT * c.MF : (yi + 1) * c.DT * c.MF],
                            ytf[yc, yl].ap()[mch],
                        )

                    def yslice(yc, yl, ki):
                        yi = ysl_idx[yc, yl]
                        return ytile[:, yi * c.DT * c.MF + ki * c.MF : yi * c.DT * c.MF + (ki + 1) * c.MF]

                    for qt in range(c.QTS):
                        zp = ps.tile([P, 512], FP32, tag="ps", bufs=6)
                        zacc = zp[:, : c.MF]
                        nmm = 2 * c.DT * 3
                        i = 0
                        for comp, yc in (("re", "re"), ("nim", "im")):
                            for ki in range(c.DT):
                                for ql, yl in (("h", "h"), ("h", "l"), ("l", "h")):
                                    nc.tensor.matmul(
                                        zacc,
                                        q2_sb[comp, ql][:, ki * c.NL + qt * P : ki * c.NL + (qt + 1) * P],
                                        yslice(yc, yl, ki),
                                        start=(i == 0),
                                        stop=(i == nmm - 1),
                                    )
                                    i += 1
                        nc.vector.reduce_max(cm[qt][:, mch : mch + 1], zacc, axis=X)
                        nc.vector.tensor_scalar_mul(
                            ncm[qt][:, mch : mch + 1], cm[qt][:, mch : mch + 1], -1.0
                        )
                        nc.scalar.activation(
                            p_sb[qt][:, mch * c.MF : (mch + 1) * c.MF],
                            zacc,
                            mybir.ActivationFunctionType.Exp,
                            bias=ncm[qt][:, mch : mch + 1],
                            scale=1.0,
                        )

                # ---------- finalize softmax: rescale chunks to global max
                for qt in range(c.QTS):
                    ngm = scp.tile([P, 1], FP32, tag=f"ngm{qt}", name=f"ngm{qt}_{rep}")
                    nc.vector.tensor_reduce(
                        ngm[:], ncm[qt][:], op=mybir.AluOpType.min, axis=X
                    )
                    fac = scp.tile([P, c.MCH], FP32, tag=f"fac{qt}", name=f"fac{qt}_{rep}")
                    nc.scalar.activation(
                        fac[:],
                        ncm[qt][:],
                        mybir.ActivationFunctionType.Exp,
                        bias=ngm[:, 0:1],
                        scale=-1.0,
                    )
                    for mch in range(c.MCH):
                        nc.vector.tensor_scalar_mul(
                            p_sb[qt][:, mch * c.MF : (mch + 1) * c.MF],
                            p_sb[qt][:, mch * c.MF : (mch + 1) * c.MF],
                            fac[:, mch : mch + 1],
                        )
                    ssum = scp.tile([P, 1], FP32, tag=f"ssum{qt}")
                    nc.vector.reduce_sum(ssum[:], p_sb[qt][:], axis=X)
                    nc.vector.reciprocal(recip[qt][:], ssum[:])
                scp.release()
                if stop_after == "scores":
                    L.release()
                    return

                # ---------- transpose P -> P^T tiles ([m-part, q-free])
                # pt_sb aliases the q2 + rt_s slots: 4 flat tiles, 8 mtg-chunks each
                pt_flat = [Lt(tag) for tag in
                           ("q_re_h", "q_re_l", "rt_s_h", "rt_s_l")]

                def pt_slice(mtg, qt=None):
                    base = (mtg % 8) * c.NL
                    if qt is None:
                        return pt_flat[mtg // 8][:, base : base + c.NL]
                    return pt_flat[mtg // 8][:, base + qt * P : base + (qt + 1) * P]

                avp = tc.alloc_tile_pool(name=f"avp{rep}", bufs=1)
                for mtg in range(c.MTG):
                    tp = ps.tile([P, 512], FP16, tag="dsc", bufs=2)
                    tacc = tp[:, : c.NL]
                    for qt in range(c.QTS):
                        nc.tensor.matmul(
                            tacc[:, qt * P : (qt + 1) * P],
                            p_sb[qt][:, mtg * P : (mtg + 1) * P],
                            ident_sb[:],
                            start=True,
                            stop=True,
                            is_transpose=True,
                        )
                    nc.vector.tensor_copy(pt_slice(mtg), tacc)
                if stop_after == "transp":
                    avp.release()
                    L.release()
                    return

                # ---------- A @ V (+ 1/sum scaling)
                for comp, odram in (("re", o_re), ("im", o_im)):
                    si = 0 if comp == "re" else 1
                    for dch in range(c.DCH):
                        vh = avp.tile([P, c.MTG * c.DF], FP16, tag="vh", bufs=2)
                        for r in range(c.NC):
                            src = agv_out[
                                r * 2 * c.SLOT + si * c.SLOT : r * 2 * c.SLOT + (si + 1) * c.SLOT
                            ].rearrange("(m p dc d) -> dc p m d", m=c.MTS, p=P, dc=c.DCH)[dch]
                            nc.sync.dma_start(
                                vh[
                                    :, r * c.MTS * c.DF : (r + 1) * c.MTS * c.DF
                                ].rearrange("p (m d) -> p m d", m=c.MTS),
                                src,
                            )
                        for qt in range(c.QTS):
                            op_ = ps.tile([P, 512], FP32, tag="ps", bufs=6)
                            oacc = op_[:, : c.DF]
                            for mtg in range(c.MTG):
                                nc.tensor.matmul(
                                    oacc,
                                    pt_slice(mtg, qt),
                                    vh[:, mtg * c.DF : (mtg + 1) * c.DF],
                                    start=(mtg == 0),
                                    stop=(mtg == c.MTG - 1),
                                )
                            osb = avp.tile([P, c.DF], FP32, tag="osb", bufs=4)
                            nc.vector.tensor_scalar_mul(osb[:], oacc, recip[qt][:, 0:1])
                            nc.sync.dma_start(
                                odram.ap()[
                                    qt * P : (qt + 1) * P, dch * c.DF : (dch + 1) * c.DF
                                ],
                                osb[:],
                            )
                avp.release()
                L.release()

            for rep in range(reps):
                emit(rep)

    nc.compile()
    return nc


def _split16(x):
    h = x.astype(np.float16)
    l = (x - h.astype(np.float32)).astype(np.float16)
    return h, l


def prep_inputs(cfg, R_re, R_im, Y_re, Y_im, W_Q_re, W_Q_im, W_K_re, W_K_im, W_V_re, W_V_im):
    """Host-side sharding + fp16 hi/lo split + transposes. Returns in_maps."""
    c = cfg
    f32 = np.float32
    f64 = np.float64
    DT, DCH, DF, MCH, MF = c.DT, c.DCH, c.DF, c.MCH, c.MF

    def _wsw(w16, ocols):
        # [d_in, d_out] -> [d_out_block, p, d_in_tile * ocols], contiguous
        ob = w16.shape[1] // ocols
        return np.ascontiguousarray(
            w16.reshape(DT, P, ob, ocols).transpose(2, 1, 0, 3).reshape(ob, P, DT * ocols)
        )

    # fused query-side weight: G = BETA * conj(W_Q) @ W_K^T  (host, fp64)
    A = np.asarray(W_Q_re, dtype=f64)
    B = np.asarray(W_Q_im, dtype=f64)
    C = np.asarray(W_K_re, dtype=f64)
    D = np.asarray(W_K_im, dtype=f64)
    g_re = (BETA * (A @ C.T + B @ D.T)).astype(f32)
    g_im = (BETA * (A @ D.T - B @ C.T)).astype(f32)
    gs = {"re": _split16(g_re), "im": _split16(g_im), "d": _split16(g_re - g_im)}

    wv_re = np.ascontiguousarray(W_V_re, dtype=f32)
    wv_im = np.ascontiguousarray(W_V_im, dtype=f32)

    shared = {}
    for comp in ("re", "im", "d"):
        for li, lvl in enumerate(("h", "l")):
            shared[f"g_{comp}_{lvl}"] = _wsw(gs[comp][li], P)
    shared["wv_re"] = _wsw(wv_re.astype(np.float16), DF)
    shared["wv_im"] = _wsw(wv_im.astype(np.float16), DF)
    shared["wv_s"] = _wsw((wv_re + wv_im).astype(np.float16), DF)
    shared["ident"] = np.eye(P, dtype=np.float16)

    # full Y^T hi/lo for scores: [MCH, P, DT*MF]
    for comp, arr in (("re", Y_re), ("im", Y_im)):
        t = np.ascontiguousarray(np.asarray(arr, dtype=f32).T)
        for lvl, a in zip(("h", "l"), _split16(t)):
            shared[f"ytf_{comp}_{lvl}"] = np.ascontiguousarray(
                a.reshape(DT, P, MCH, MF).transpose(2, 1, 0, 3).reshape(MCH, P, DT * MF)
            )

    in_maps = []
    for r in range(c.NC):
        m = dict(shared)
        rsl = slice(r * c.NL, (r + 1) * c.NL)
        ysl = slice(r * c.ML, (r + 1) * c.ML)
        rre_t = np.ascontiguousarray(np.asarray(R_re[rsl], dtype=f32).T)
        rim_t = np.ascontiguousarray(np.asarray(R_im[rsl], dtype=f32).T)
        yre_t = np.ascontiguousarray(np.asarray(Y_re[ysl], dtype=f32).T)
        yim_t = np.ascontiguousarray(np.asarray(Y_im[ysl], dtype=f32).T)
        for base, arr in (("rt_re", rre_t), ("rt_im", rim_t), ("rt_s", rre_t + rim_t)):
            h, l = _split16(arr)
            mw = arr.shape[1]
            for lvl, a in (("h", h), ("l", l)):
                m[f"{base}_{lvl}"] = np.ascontiguousarray(
                    a.reshape(DT, P, mw).transpose(1, 0, 2).reshape(P, DT * mw)
                )
        for n, arr in (("re", yre_t), ("im", yim_t), ("s", yre_t + yim_t)):
            mw = arr.shape[1]
            m[f"ytl_{n}"] = np.ascontiguousarray(
                arr.astype(np.float16).reshape(DT, P, mw).transpose(1, 0, 2).reshape(P, DT * mw)
            )
        in_maps.append(m)
    return in_maps


_NC_CACHE = {}


def kernel(**inputs) -> np.ndarray:
    cfg = Cfg()
    if "full" not in _NC_CACHE:
        _NC_CACHE["full"] = build(cfg, 1)
    nc = _NC_CACHE["full"]
    in_maps = prep_inputs(cfg, **inputs)
    res = run_bass_kernel_spmd(nc, in_maps, list(range(cfg.NC)))
    o_re = np.concatenate([res.results[r]["o_re"] for r in range(cfg.NC)], axis=0)
    o_im = np.concatenate([res.results[r]["o_im"] for r in range(cfg.NC)], axis=0)
    return (o_re + 1j * o_im).astype(np.complex64)
